# revision 5
# baseline (speedup 1.0000x reference)
"""Trainium2 Bass kernel for nn_Llama_head (paired two-tower MLP head).

Computes sigmoid(rowwise_dot(mlp_u(xu), mlp_i(xv))) for N=32768 rows,
data-parallel across 8 NeuronCores (N sharded, weights replicated).

The host pre-packs x into bf16 tiles laid out exactly as the PE wants
its moving operand ([128 d-partitions, k-tile, n]), so the kernel has
no on-chip transposes and half the HBM traffic of an f32 upload.

Per-core structure (Nc = 4096 rows, blocks of NB = 512 rows):
  - DMA rings: xu blocks on the sync HWDGE ring, xv blocks on the
    scalar HWDGE ring (hides the per-DMA completion-receipt gap each
    ring imposes), weights/constants/outputs on the gpsimd SWDGE ring.
  - A dozen warm-up matmuls on a memset scratch tile run while the
    first x bytes stream in, flipping the PE HAM clock gate to 2.4GHz
    before real work starts.
  - Layer 1: hT[h, n] += w1[dk, h].T @ xT[dk, n], 32 k-tiles into PSUM.
  - ACT relu (+b1) PSUM->SBUF bf16; layer 2 + row-dot matmuls are
    emitted a few L1 matmuls *into the next tower's stream* so the PE
    never waits on the ACT/DVE round trips.
  - DVE: u = uT + b2; prod = u * v; PE: ones.T @ prod -> diag[1, n];
    ACT sigmoid -> s_blk; per-block 2KB DMA to out.
"""

import os

import numpy as np
import ml_dtypes

# Problem shape (hardcoded per harness contract).
N_FULL = 32768
D = 4096
H = 256
O = 64
N_CORES = 8

NC_ROWS = N_FULL // N_CORES  # rows per core
NB = 512                     # rows per block
NBLK = NC_ROWS // NB
KT = D // 128                # layer-1 k-tiles
HH_T = H // 128              # layer-2 k-tiles (= layer-1 out tiles)
N_WARM = 12                  # HAM warm-up dummy matmuls
TRACE = bool(int(os.environ.get("KERNEL_TRACE", "0")))

LAST_RESULTS = None  # BassKernelResults of the most recent run (for profiling)

_PROGRAM = None


def _build_program():
    from contextlib import ExitStack

    import concourse.mybir as mybir
    import concourse.tile as tile
    from concourse import bacc

    f32 = mybir.dt.float32
    bf16 = mybir.dt.bfloat16
    AF = mybir.ActivationFunctionType

    nc = bacc.Bacc("TRN2")

    xu = nc.dram_tensor("xu", [NBLK, 128, KT, NB], bf16, kind="ExternalInput")
    xv = nc.dram_tensor("xv", [NBLK, 128, KT, NB], bf16, kind="ExternalInput")
    w1u = nc.dram_tensor("w1u", [128, KT, H], bf16, kind="ExternalInput")
    w1i = nc.dram_tensor("w1i", [128, KT, H], bf16, kind="ExternalInput")
    w2u = nc.dram_tensor("w2u", [128, HH_T, O], bf16, kind="ExternalInput")
    w2i = nc.dram_tensor("w2i", [128, HH_T, O], bf16, kind="ExternalInput")
    cst_d = nc.dram_tensor("cst", [128, 6], f32, kind="ExternalInput")
    ones_d = nc.dram_tensor("ones", [O, 1], bf16, kind="ExternalInput")
    out = nc.dram_tensor("out", [NC_ROWS], f32, kind="ExternalOutput")

    with ExitStack() as ctx:
        tc = ctx.enter_context(tile.TileContext(nc))

        wpool = ctx.enter_context(tc.tile_pool(name="weights", bufs=1))
        xp = ctx.enter_context(tc.tile_pool(name="x", bufs=4))
        hp = ctx.enter_context(tc.tile_pool(name="h", bufs=4))
        uvp = ctx.enter_context(tc.tile_pool(name="uv", bufs=4))
        sp = ctx.enter_context(tc.tile_pool(name="sblk", bufs=2))
        ps_h = ctx.enter_context(tc.tile_pool(name="psh", bufs=4, space="PSUM"))
        ps_uv = ctx.enter_context(tc.tile_pool(name="psuv", bufs=2, space="PSUM"))
        ps_d = ctx.enter_context(tc.tile_pool(name="psd", bufs=2, space="PSUM"))

        # --- PE warm-up: matmuls on a memset scratch tile keep the PE's
        # HAM activity window busy from ~0.5us so the 2.4GHz clock gate is
        # open by the time the first real operands land.
        scratch = wpool.tile([128, 128 + NB], bf16, tag="scr", name="scratch")
        nc.gpsimd.memset(scratch, 0)
        ph_warm = ps_h.tile([128, NB], f32, tag="ph", name="ph_warm")
        for _ in range(N_WARM):
            nc.tensor.matmul(
                ph_warm, scratch[:, 0:128], scratch[:, 128:], start=True, stop=True
            )

        # --- weights / constants on the gpsimd SWDGE ring, chunked and
        # ordered by first use so early x DMAs aren't starved of HBM BW.
        w1_sb = {}
        w2_sb = {}
        for name, w1d in (("u", w1u), ("i", w1i)):
            w1_sb[name] = wpool.tile([128, KT, H], bf16, tag=f"w1{name}", name=f"w1{name}")
        for name, w2d in (("u", w2u), ("i", w2i)):
            w2_sb[name] = wpool.tile([128, HH_T, O], bf16, tag=f"w2{name}", name=f"w2{name}")
        cst = wpool.tile([128, 6], f32, tag="cst", name="cst")
        ones_sb = wpool.tile([O, 1], bf16, tag="ones", name="ones")

        nc.gpsimd.dma_start(w1_sb["u"][:, 0:12, :], w1u[:, 0:12, :])
        nc.gpsimd.dma_start(w1_sb["i"][:, 0:12, :], w1i[:, 0:12, :])
        nc.gpsimd.dma_start(cst, cst_d[:])
        nc.gpsimd.dma_start(ones_sb, ones_d[:])
        nc.gpsimd.dma_start(w1_sb["u"][:, 12:32, :], w1u[:, 12:32, :])
        nc.gpsimd.dma_start(w1_sb["i"][:, 12:32, :], w1i[:, 12:32, :])
        nc.gpsimd.dma_start(w2_sb["u"], w2u[:])
        nc.gpsimd.dma_start(w2_sb["i"], w2i[:])

        b1_sb = {"u": cst[:, 0:2], "i": cst[:, 2:4]}
        b2_sb = {"u": cst[:O, 4:5], "i": cst[:O, 5:6]}

        # --- x streams: xu on sync ring, xv on scalar ring.
        dma_eng = {"u": nc.sync, "i": nc.scalar}
        x_d = {"u": xu, "i": xv}

        # Block 0 in two chunks so the first matmuls start early; later
        # blocks are single 4MB transfers (ring-gap amortized).
        nat_first = {}
        for sname in ("u", "i"):
            nat = xp.tile([128, KT, NB], bf16, tag="x", name="x")
            for q0, q1 in ((0, 12), (12, 32)):
                dma_eng[sname].dma_start(nat[:, q0:q1, :], x_d[sname][0][:, q0:q1, :])
            nat_first[sname] = nat

        # --- deferred PE emission: L2/dot matmuls are queued and emitted
        # a few L1 matmuls into the *following* stream so the PE never
        # stalls on the ACT relu / DVE round trips they depend on.
        pending = []

        def after_mms(n, fn):
            pending.append([n, fn])

        def tick():
            due = [it for it in pending if it[0] <= 1]
            for it in due:
                pending.remove(it)
                it[1]()
            for it in pending:
                it[0] -= 1

        def flush():
            while pending:
                it = pending.pop(0)
                it[1]()

        for b in range(NBLK):
            stash = {}
            for sname in ("u", "i"):
                if b == 0:
                    nat = nat_first[sname]
                else:
                    nat = xp.tile([128, KT, NB], bf16, tag="x", name="x")
                    dma_eng[sname].dma_start(nat, x_d[sname][b])

                ph = [ps_h.tile([128, NB], f32, tag="ph", name="ph") for _ in range(HH_T)]
                hsb = [hp.tile([128, NB], bf16, tag="h", name="h") for _ in range(HH_T)]
                for hh in range(HH_T):
                    for k in range(KT):
                        nc.tensor.matmul(
                            ph[hh],
                            w1_sb[sname][:, k, hh * 128 : (hh + 1) * 128],
                            nat[:, k, :],
                            start=(k == 0),
                            stop=(k == KT - 1),
                        )
                        tick()
                    # relu as soon as this hh's accumulation group closes
                    nc.scalar.activation(
                        hsb[hh], ph[hh], AF.Relu, bias=b1_sb[sname][:, hh : hh + 1]
                    )

                def l2(sname=sname, hsb=hsb, stash=stash):
                    puv = ps_uv.tile([O, NB], f32, tag="puv", name="puv")
                    for hh in range(HH_T):
                        nc.tensor.matmul(
                            puv,
                            w2_sb[sname][:, hh, :],
                            hsb[hh],
                            start=(hh == 0),
                            stop=(hh == HH_T - 1),
                        )
                    usb = uvp.tile([O, NB], bf16, tag="uv", name="uv")
                    nc.vector.tensor_scalar_add(usb, puv, b2_sb[sname])
                    stash[sname] = usb

                after_mms(3, l2)

            def dot(b=b, stash=stash):
                prod = uvp.tile([O, NB], bf16, tag="prod", name="prod")
                nc.vector.tensor_mul(prod, stash["u"], stash["i"])
                pd = ps_d.tile([1, NB], f32, tag="pd", name="pd")
                nc.tensor.matmul(pd, ones_sb, prod, start=True, stop=True)
                s_blk = sp.tile([1, NB], f32, tag="sblk", name="s_blk")
                nc.scalar.activation(s_blk, pd, AF.Sigmoid)
                nc.gpsimd.dma_start(out[b * NB : (b + 1) * NB], s_blk)

            after_mms(6, dot)

        flush()

    nc.compile()
    return nc


def _pack_x(x):
    """[N_FULL, D] f32 -> per-core [NBLK, 128, KT, NB] bf16 tiles.

    packed[c][b, p, k, n] = x[c*NC_ROWS + b*NB + n, k*128 + p]
    """
    xb = np.asarray(x, dtype=np.float32).astype(ml_dtypes.bfloat16)
    xb = xb.reshape(N_CORES, NBLK, NB, KT, 128)
    return [np.ascontiguousarray(xb[c].transpose(0, 3, 2, 1)) for c in range(N_CORES)]


def _pack_w1(w1):
    """[D, H] -> [128, KT, H] bf16: element (p, k, h) = w1[k*128+p, h]."""
    wb = np.asarray(w1, dtype=np.float32).astype(ml_dtypes.bfloat16)
    return np.ascontiguousarray(wb.reshape(KT, 128, H).transpose(1, 0, 2))


def _pack_w2(w2):
    """[H, O] -> [128, HH_T, O] bf16: element (p, a, o) = w2[a*128+p, o]."""
    wb = np.asarray(w2, dtype=np.float32).astype(ml_dtypes.bfloat16)
    return np.ascontiguousarray(wb.reshape(HH_T, 128, O).transpose(1, 0, 2))


def _pack_cst(b1u, b1i, b2u, b2i):
    """[128, 6] f32: b1u as 2 cols, b1i as 2 cols, b2u, b2i (zero-padded)."""
    cst = np.zeros((128, 6), dtype=np.float32)
    cst[:, 0:2] = b1u.reshape(2, 128).T
    cst[:, 2:4] = b1i.reshape(2, 128).T
    cst[: b2u.shape[0], 4] = b2u
    cst[: b2i.shape[0], 5] = b2i
    return cst


def _get_program():
    global _PROGRAM
    if _PROGRAM is None:
        _PROGRAM = _build_program()
    return _PROGRAM


def kernel(
    user_origin_emb,
    item_origin_emb,
    u_w1,
    u_b1,
    u_w2,
    u_b2,
    i_w1,
    i_b1,
    i_w2,
    i_b2,
):
    global LAST_RESULTS
    from concourse.bass_utils import run_bass_kernel_spmd

    xu_packed = _pack_x(user_origin_emb)
    xv_packed = _pack_x(item_origin_emb)
    shared = {
        "w1u": _pack_w1(u_w1),
        "w1i": _pack_w1(i_w1),
        "w2u": _pack_w2(u_w2),
        "w2i": _pack_w2(i_w2),
        "cst": _pack_cst(
            np.asarray(u_b1, dtype=np.float32),
            np.asarray(i_b1, dtype=np.float32),
            np.asarray(u_b2, dtype=np.float32),
            np.asarray(i_b2, dtype=np.float32),
        ),
        "ones": np.ones((O, 1), dtype=ml_dtypes.bfloat16),
    }

    nc = _get_program()
    in_maps = [
        {"xu": xu_packed[c], "xv": xv_packed[c], **shared}
        for c in range(N_CORES)
    ]
    res = run_bass_kernel_spmd(nc, in_maps, core_ids=list(range(N_CORES)), trace=TRACE)
    LAST_RESULTS = res
    return np.concatenate([r["out"] for r in res.results], axis=0)


# revision 9
# speedup vs baseline: 1.0729x; 1.0729x over previous
"""Trainium2 Bass kernel for nn_Llama_head (paired two-tower MLP head).

Computes sigmoid(rowwise_dot(mlp_u(xu), mlp_i(xv))) for N=32768 rows,
data-parallel across 8 NeuronCores (N sharded, weights replicated).

The host pre-packs x into bf16 tiles laid out exactly as the PE wants
its moving operand ([128 d-partitions, k-tile, n]), so the kernel has
no on-chip transposes and half the HBM traffic of an f32 upload.

Per-core structure (Nc = 4096 rows, blocks of NB = 512 rows):
  - DMA rings: xu blocks on the sync HWDGE ring, xv blocks on the
    scalar HWDGE ring (hides the per-DMA completion-receipt gap each
    ring imposes), weights/constants/outputs on the gpsimd SWDGE ring.
  - A dozen warm-up matmuls on a memset scratch tile run while the
    first x bytes stream in, flipping the PE HAM clock gate to 2.4GHz
    before real work starts.
  - Layer 1: hT[h, n] += w1[dk, h].T @ xT[dk, n], 32 k-tiles into PSUM.
  - ACT relu (+b1) PSUM->SBUF bf16; layer 2 + row-dot matmuls are
    emitted a few L1 matmuls *into the next tower's stream* so the PE
    never waits on the ACT/DVE round trips.
  - DVE: u = uT + b2; prod = u * v; PE: ones.T @ prod -> diag[1, n];
    ACT sigmoid -> s_blk; per-block 2KB DMA to out.
"""

import os

import numpy as np
import ml_dtypes

# Problem shape (hardcoded per harness contract).
N_FULL = 32768
D = 4096
H = 256
O = 64
N_CORES = 8

NC_ROWS = N_FULL // N_CORES  # rows per core
NB = 512                     # rows per block
NBLK = NC_ROWS // NB
KT = D // 128                # layer-1 k-tiles
HH_T = H // 128              # layer-2 k-tiles (= layer-1 out tiles)
N_WARM = 14                  # HAM warm-up dummy matmuls
TRACE = bool(int(os.environ.get("KERNEL_TRACE", "0")))

LAST_RESULTS = None  # BassKernelResults of the most recent run (for profiling)

_PROGRAM = None


def _build_program():
    from contextlib import ExitStack

    import concourse.mybir as mybir
    import concourse.tile as tile
    from concourse import bacc

    f32 = mybir.dt.float32
    bf16 = mybir.dt.bfloat16
    AF = mybir.ActivationFunctionType

    nc = bacc.Bacc("TRN2")

    xu = nc.dram_tensor("xu", [NBLK, 128, KT, NB], bf16, kind="ExternalInput")
    xv = nc.dram_tensor("xv", [NBLK, 128, KT, NB], bf16, kind="ExternalInput")
    w1u = nc.dram_tensor("w1u", [128, KT, H], bf16, kind="ExternalInput")
    w1i = nc.dram_tensor("w1i", [128, KT, H], bf16, kind="ExternalInput")
    w2u = nc.dram_tensor("w2u", [128, HH_T, O], bf16, kind="ExternalInput")
    w2i = nc.dram_tensor("w2i", [128, HH_T, O], bf16, kind="ExternalInput")
    cst_d = nc.dram_tensor("cst", [128, 6], f32, kind="ExternalInput")
    ones_d = nc.dram_tensor("ones", [O, 1], bf16, kind="ExternalInput")
    out = nc.dram_tensor("out", [NC_ROWS], f32, kind="ExternalOutput")

    with ExitStack() as ctx:
        tc = ctx.enter_context(tile.TileContext(nc))

        wpool = ctx.enter_context(tc.tile_pool(name="weights", bufs=1))
        xp = ctx.enter_context(tc.tile_pool(name="x", bufs=4))
        hp = ctx.enter_context(tc.tile_pool(name="h", bufs=4))
        uvp = ctx.enter_context(tc.tile_pool(name="uv", bufs=4))
        sp = ctx.enter_context(tc.tile_pool(name="sblk", bufs=2))
        ps_h = ctx.enter_context(tc.tile_pool(name="psh", bufs=4, space="PSUM"))
        ps_uv = ctx.enter_context(tc.tile_pool(name="psuv", bufs=2, space="PSUM"))
        ps_d = ctx.enter_context(tc.tile_pool(name="psd", bufs=2, space="PSUM"))

        # --- PE warm-up: matmuls on a memset scratch tile (result is
        # never read) keep the PE's HAM activity window busy from engine
        # start so the 2.4GHz clock gate is open when real operands land.
        scratch = wpool.tile([128, 128 + NB], bf16, tag="scr", name="scratch")
        nc.vector.memset(scratch, 0)
        ph_warm = ps_h.tile([128, NB], f32, tag="ph", name="ph_warm")
        for _ in range(N_WARM):
            nc.tensor.matmul(
                ph_warm, scratch[:, 0:128], scratch[:, 128:], start=True, stop=True
            )

        w1_sb = {}
        w2_sb = {}
        for name in ("u", "i"):
            w1_sb[name] = wpool.tile([128, KT, H], bf16, tag=f"w1{name}", name=f"w1{name}")
            w2_sb[name] = wpool.tile([128, HH_T, O], bf16, tag=f"w2{name}", name=f"w2{name}")
        cst = wpool.tile([128, 6], f32, tag="cst", name="cst")
        ones_sb = wpool.tile([O, 1], bf16, tag="ones", name="ones")

        # Tiny constants on the gpsimd SWDGE ring (drains instantly, then
        # the ring only carries the per-block 2KB output writes).
        nc.gpsimd.dma_start(cst, cst_d[:])
        nc.gpsimd.dma_start(ones_sb, ones_d[:])
        nc.gpsimd.dma_start(w2_sb["u"], w2u[:])
        nc.gpsimd.dma_start(w2_sb["i"], w2i[:])

        b1_sb = {"u": cst[:, 0:2], "i": cst[:, 2:4]}
        b2_sb = {"u": cst[:O, 4:5], "i": cst[:O, 5:6]}

        # --- x + w1 streams, ping-pong striped across the two HWDGE
        # rings: sync carries k-tiles [0:16) of every (w1, x) item in
        # consumption order, scalar carries k-tiles [16:32). While the PE
        # eats one ring's half the other ring delivers the next half, so
        # the in-order consumption stream gets both rings' bandwidth.
        KH = KT // 2
        w1_d = {"u": w1u, "i": w1i}
        x_d = {"u": xu, "i": xv}

        # w1u low half finely chunked ahead of x so the very first matmul
        # only waits on ~0.75MB.
        nc.sync.dma_start(w1_sb["u"][:, 0:4, :], w1u[:, 0:4, :])
        nc.scalar.dma_start(w1_sb["u"][:, KH:, :], w1u[:, KH:, :])
        nat_first = {}
        for sname in ("u", "i"):
            nat_first[sname] = xp.tile([128, KT, NB], bf16, tag="x", name="x")
        nc.sync.dma_start(nat_first["u"][:, 0:4, :], xu[0][:, 0:4, :])
        nc.sync.dma_start(w1_sb["u"][:, 4:KH, :], w1u[:, 4:KH, :])
        nc.sync.dma_start(nat_first["u"][:, 4:KH, :], xu[0][:, 4:KH, :])
        nc.scalar.dma_start(nat_first["u"][:, KH:, :], xu[0][:, KH:, :])
        nc.sync.dma_start(w1_sb["i"][:, 0:KH, :], w1i[:, 0:KH, :])
        nc.scalar.dma_start(w1_sb["i"][:, KH:, :], w1i[:, KH:, :])
        nc.sync.dma_start(nat_first["i"][:, 0:KH, :], xv[0][:, 0:KH, :])
        nc.scalar.dma_start(nat_first["i"][:, KH:, :], xv[0][:, KH:, :])

        # --- deferred PE emission: L2/dot matmuls are queued and emitted
        # a few L1 matmuls into the *following* stream so the PE never
        # stalls on the ACT relu / DVE round trips they depend on.
        pending = []

        def after_mms(n, fn):
            pending.append([n, fn])

        def tick():
            due = [it for it in pending if it[0] <= 1]
            for it in due:
                pending.remove(it)
                it[1]()
            for it in pending:
                it[0] -= 1

        def flush():
            while pending:
                it = pending.pop(0)
                it[1]()

        for b in range(NBLK):
            stash = {}
            for sname in ("u", "i"):
                if b == 0:
                    nat = nat_first[sname]
                else:
                    nat = xp.tile([128, KT, NB], bf16, tag="x", name="x")
                    nc.sync.dma_start(nat[:, 0:KH, :], x_d[sname][b][:, 0:KH, :])
                    nc.scalar.dma_start(nat[:, KH:, :], x_d[sname][b][:, KH:, :])

                ph = [ps_h.tile([128, NB], f32, tag="ph", name="ph") for _ in range(HH_T)]
                hsb = [hp.tile([128, NB], bf16, tag="h", name="h") for _ in range(HH_T)]
                for hh in range(HH_T):
                    for k in range(KT):
                        nc.tensor.matmul(
                            ph[hh],
                            w1_sb[sname][:, k, hh * 128 : (hh + 1) * 128],
                            nat[:, k, :],
                            start=(k == 0),
                            stop=(k == KT - 1),
                        )
                        tick()
                    # relu as soon as this hh's accumulation group closes
                    nc.scalar.activation(
                        hsb[hh], ph[hh], AF.Relu, bias=b1_sb[sname][:, hh : hh + 1]
                    )

                def l2(sname=sname, hsb=hsb, stash=stash):
                    puv = ps_uv.tile([O, NB], f32, tag="puv", name="puv")
                    for hh in range(HH_T):
                        nc.tensor.matmul(
                            puv,
                            w2_sb[sname][:, hh, :],
                            hsb[hh],
                            start=(hh == 0),
                            stop=(hh == HH_T - 1),
                        )
                    usb = uvp.tile([O, NB], bf16, tag="uv", name="uv")
                    nc.vector.tensor_scalar_add(usb, puv, b2_sb[sname])
                    stash[sname] = usb

                after_mms(3, l2)

            def dot(b=b, stash=stash):
                prod = uvp.tile([O, NB], bf16, tag="prod", name="prod")
                nc.vector.tensor_mul(prod, stash["u"], stash["i"])
                pd = ps_d.tile([1, NB], f32, tag="pd", name="pd")
                nc.tensor.matmul(pd, ones_sb, prod, start=True, stop=True)
                s_blk = sp.tile([1, NB], f32, tag="sblk", name="s_blk")
                nc.scalar.activation(s_blk, pd, AF.Sigmoid)
                nc.gpsimd.dma_start(out[b * NB : (b + 1) * NB], s_blk)

            after_mms(6, dot)

        flush()

    nc.compile()
    return nc


def _pack_x(x):
    """[N_FULL, D] f32 -> per-core [NBLK, 128, KT, NB] bf16 tiles.

    packed[c][b, p, k, n] = x[c*NC_ROWS + b*NB + n, k*128 + p]
    """
    xb = np.asarray(x, dtype=np.float32).astype(ml_dtypes.bfloat16)
    xb = xb.reshape(N_CORES, NBLK, NB, KT, 128)
    return [np.ascontiguousarray(xb[c].transpose(0, 3, 2, 1)) for c in range(N_CORES)]


def _pack_w1(w1):
    """[D, H] -> [128, KT, H] bf16: element (p, k, h) = w1[k*128+p, h]."""
    wb = np.asarray(w1, dtype=np.float32).astype(ml_dtypes.bfloat16)
    return np.ascontiguousarray(wb.reshape(KT, 128, H).transpose(1, 0, 2))


def _pack_w2(w2):
    """[H, O] -> [128, HH_T, O] bf16: element (p, a, o) = w2[a*128+p, o]."""
    wb = np.asarray(w2, dtype=np.float32).astype(ml_dtypes.bfloat16)
    return np.ascontiguousarray(wb.reshape(HH_T, 128, O).transpose(1, 0, 2))


def _pack_cst(b1u, b1i, b2u, b2i):
    """[128, 6] f32: b1u as 2 cols, b1i as 2 cols, b2u, b2i (zero-padded)."""
    cst = np.zeros((128, 6), dtype=np.float32)
    cst[:, 0:2] = b1u.reshape(2, 128).T
    cst[:, 2:4] = b1i.reshape(2, 128).T
    cst[: b2u.shape[0], 4] = b2u
    cst[: b2i.shape[0], 5] = b2i
    return cst


def _get_program():
    global _PROGRAM
    if _PROGRAM is None:
        _PROGRAM = _build_program()
    return _PROGRAM


def kernel(
    user_origin_emb,
    item_origin_emb,
    u_w1,
    u_b1,
    u_w2,
    u_b2,
    i_w1,
    i_b1,
    i_w2,
    i_b2,
):
    global LAST_RESULTS
    from concourse.bass_utils import run_bass_kernel_spmd

    xu_packed = _pack_x(user_origin_emb)
    xv_packed = _pack_x(item_origin_emb)
    shared = {
        "w1u": _pack_w1(u_w1),
        "w1i": _pack_w1(i_w1),
        "w2u": _pack_w2(u_w2),
        "w2i": _pack_w2(i_w2),
        "cst": _pack_cst(
            np.asarray(u_b1, dtype=np.float32),
            np.asarray(i_b1, dtype=np.float32),
            np.asarray(u_b2, dtype=np.float32),
            np.asarray(i_b2, dtype=np.float32),
        ),
        "ones": np.ones((O, 1), dtype=ml_dtypes.bfloat16),
    }

    nc = _get_program()
    in_maps = [
        {"xu": xu_packed[c], "xv": xv_packed[c], **shared}
        for c in range(N_CORES)
    ]
    res = run_bass_kernel_spmd(nc, in_maps, core_ids=list(range(N_CORES)), trace=TRACE)
    LAST_RESULTS = res
    return np.concatenate([r["out"] for r in res.results], axis=0)


# revision 10
# speedup vs baseline: 1.0972x; 1.0226x over previous
"""Trainium2 Bass kernel for nn_Llama_head (paired two-tower MLP head).

Computes sigmoid(rowwise_dot(mlp_u(xu), mlp_i(xv))) for N=32768 rows,
data-parallel across 8 NeuronCores (N sharded, weights replicated).

Key structure:
  - Host pre-packs x transposed/tiled so the kernel has no on-chip
    transposes: bf16 for k-tiles [0,26), fp8-e4m3 for k-tiles [26,32).
    The fp8 k-tiles run as DoubleRow (double-pumped) matmuls; w1 is
    pre-scaled by 512 so its values sit in e4m3's normal range, with
    the inverse folded into b1 (x512, relu is positively homogeneous)
    and w2 (/512). Measured end-to-end error ~1.3e-2 vs the 2e-2 gate.
  - Layer 1: hT[h, n] += w1[dk, h].T @ xT[dk, n] into PSUM (26 bf16
    k-tiles + 3 DoubleRow fp8 pairs); ACT relu (+512*b1) -> bf16 h;
    layer 2 + row-dot matmuls are emitted a few L1 matmuls into the
    next tower's PE stream so the PE never waits on ACT/DVE.
  - DMA: sync HWDGE ring starts ~6us earlier than the others, so it
    carries the need-ordered startup (w1u/xu0/w1i/xv0, k-chunked) and
    even blocks; the scalar ring carries odd blocks; the gpsimd SWDGE
    ring carries tiny constants and per-block 2KB outputs.
"""

import os

import numpy as np
import ml_dtypes

# Problem shape (hardcoded per harness contract).
N_FULL = 32768
D = 4096
H = 256
O = 64
N_CORES = 8

NC_ROWS = N_FULL // N_CORES  # rows per core
NB = 512                     # rows per block
NBLK = NC_ROWS // NB
KT = D // 128                # layer-1 k-tiles
KT8 = 6                      # k-tiles computed in fp8 (DoubleRow pairs)
KTB = KT - KT8               # k-tiles computed in bf16
HH_T = H // 128              # layer-2 k-tiles (= layer-1 out tiles)
W1SCALE = 512.0              # w1 pre-scale so fp8 tiles avoid subnormals
TRACE = bool(int(os.environ.get("KERNEL_TRACE", "0")))

LAST_RESULTS = None  # BassKernelResults of the most recent run (for profiling)

_PROGRAM = None


def _build_program():
    from contextlib import ExitStack

    import concourse.mybir as mybir
    import concourse.tile as tile
    from concourse import bacc

    f32 = mybir.dt.float32
    bf16 = mybir.dt.bfloat16
    fp8 = mybir.dt.float8e4
    AF = mybir.ActivationFunctionType
    DR = mybir.MatmulPerfMode.DoubleRow

    nc = bacc.Bacc("TRN2")

    xb_d = {
        "u": nc.dram_tensor("xub", [NBLK, 128, KTB, NB], bf16, kind="ExternalInput"),
        "i": nc.dram_tensor("xvb", [NBLK, 128, KTB, NB], bf16, kind="ExternalInput"),
    }
    x8_d = {
        "u": nc.dram_tensor("xu8", [NBLK, 128, KT8, NB], fp8, kind="ExternalInput"),
        "i": nc.dram_tensor("xv8", [NBLK, 128, KT8, NB], fp8, kind="ExternalInput"),
    }
    w1b_d = {
        "u": nc.dram_tensor("w1ub", [128, KTB, H], bf16, kind="ExternalInput"),
        "i": nc.dram_tensor("w1ib", [128, KTB, H], bf16, kind="ExternalInput"),
    }
    w18_d = {
        "u": nc.dram_tensor("w1u8", [128, KT8, H], fp8, kind="ExternalInput"),
        "i": nc.dram_tensor("w1i8", [128, KT8, H], fp8, kind="ExternalInput"),
    }
    w2_d = {
        "u": nc.dram_tensor("w2u", [128, HH_T, O], bf16, kind="ExternalInput"),
        "i": nc.dram_tensor("w2i", [128, HH_T, O], bf16, kind="ExternalInput"),
    }
    cst_d = nc.dram_tensor("cst", [128, 6], f32, kind="ExternalInput")
    ones_d = nc.dram_tensor("ones", [O, 1], bf16, kind="ExternalInput")
    out = nc.dram_tensor("out", [NC_ROWS], f32, kind="ExternalOutput")

    with ExitStack() as ctx:
        tc = ctx.enter_context(tile.TileContext(nc))

        wpool = ctx.enter_context(tc.tile_pool(name="weights", bufs=1))
        xp = ctx.enter_context(tc.tile_pool(name="x", bufs=4))
        xp8 = ctx.enter_context(tc.tile_pool(name="x8", bufs=4))
        hp = ctx.enter_context(tc.tile_pool(name="h", bufs=4))
        uvp = ctx.enter_context(tc.tile_pool(name="uv", bufs=4))
        sp = ctx.enter_context(tc.tile_pool(name="sblk", bufs=2))
        ps_h = ctx.enter_context(tc.tile_pool(name="psh", bufs=4, space="PSUM"))
        ps_uv = ctx.enter_context(tc.tile_pool(name="psuv", bufs=2, space="PSUM"))
        ps_d = ctx.enter_context(tc.tile_pool(name="psd", bufs=2, space="PSUM"))

        w1b_sb = {}
        w18_sb = {}
        w2_sb = {}
        for s in ("u", "i"):
            w1b_sb[s] = wpool.tile([128, KTB, H], bf16, tag=f"w1b{s}", name=f"w1b{s}")
            w18_sb[s] = wpool.tile([128, KT8, H], fp8, tag=f"w18{s}", name=f"w18{s}")
            w2_sb[s] = wpool.tile([128, HH_T, O], bf16, tag=f"w2{s}", name=f"w2{s}")
        cst = wpool.tile([128, 6], f32, tag="cst", name="cst")
        ones_sb = wpool.tile([O, 1], bf16, tag="ones", name="ones")

        # Tiny constants on the gpsimd SWDGE ring.
        nc.gpsimd.dma_start(cst, cst_d[:])
        nc.gpsimd.dma_start(ones_sb, ones_d[:])
        nc.gpsimd.dma_start(w2_sb["u"], w2_d["u"][:])
        nc.gpsimd.dma_start(w2_sb["i"], w2_d["i"][:])

        b1_sb = {"u": cst[:, 0:2], "i": cst[:, 2:4]}
        b2_sb = {"u": cst[:O, 4:5], "i": cst[:O, 5:6]}

        nat_b = {}
        nat_8 = {}
        for s in ("u", "i"):
            nat_b[s] = xp.tile([128, KTB, NB], bf16, tag="x", name="x")
            nat_8[s] = xp8.tile([128, KT8, NB], fp8, tag="x8", name="x8")

        # Startup on the sync ring (it starts ~6us before the others),
        # k-chunked in exact consumption order.
        for s in ("u", "i"):
            nc.sync.dma_start(w1b_sb[s][:, 0:4, :], w1b_d[s][:, 0:4, :])
            nc.sync.dma_start(nat_b[s][:, 0:4, :], xb_d[s][0][:, 0:4, :])
            nc.sync.dma_start(w1b_sb[s][:, 4:13, :], w1b_d[s][:, 4:13, :])
            nc.sync.dma_start(nat_b[s][:, 4:13, :], xb_d[s][0][:, 4:13, :])
            nc.sync.dma_start(w1b_sb[s][:, 13:KTB, :], w1b_d[s][:, 13:KTB, :])
            nc.sync.dma_start(nat_b[s][:, 13:KTB, :], xb_d[s][0][:, 13:KTB, :])
            nc.sync.dma_start(w18_sb[s], w18_d[s][:])
            nc.sync.dma_start(nat_8[s], x8_d[s][0])

        # --- deferred PE emission machinery (see module docstring).
        pending = []

        def after_mms(n, fn):
            pending.append([n, fn])

        def tick():
            due = [it for it in pending if it[0] <= 1]
            for it in due:
                pending.remove(it)
                it[1]()
            for it in pending:
                it[0] -= 1

        def flush():
            while pending:
                pending.pop(0)[1]()

        for b in range(NBLK):
            stash = {}
            for sname in ("u", "i"):
                if b == 0:
                    nat = nat_b[sname]
                    nat8 = nat_8[sname]
                else:
                    # Even blocks on the sync ring, odd on the scalar ring.
                    eng = nc.sync if b % 2 == 0 else nc.scalar
                    nat = xp.tile([128, KTB, NB], bf16, tag="x", name="x")
                    eng.dma_start(nat, xb_d[sname][b])
                    nat8 = xp8.tile([128, KT8, NB], fp8, tag="x8", name="x8")
                    eng.dma_start(nat8, x8_d[sname][b])

                ph = [ps_h.tile([128, NB], f32, tag="ph", name="ph") for _ in range(HH_T)]
                hsb = [hp.tile([128, NB], bf16, tag="h", name="h") for _ in range(HH_T)]
                for hh in range(HH_T):
                    hs = slice(hh * 128, (hh + 1) * 128)
                    for k in range(KTB):
                        nc.tensor.matmul(
                            ph[hh],
                            w1b_sb[sname][:, k, hs],
                            nat[:, k, :],
                            start=(k == 0),
                            stop=False,
                        )
                        tick()
                    for j in range(KT8 // 2):
                        nc.tensor.matmul(
                            ph[hh],
                            w18_sb[sname][:, 2 * j : 2 * j + 2, hs],
                            nat8[:, 2 * j : 2 * j + 2, :],
                            start=False,
                            stop=(j == KT8 // 2 - 1),
                            perf_mode=DR,
                        )
                        tick()
                    nc.scalar.activation(
                        hsb[hh], ph[hh], AF.Relu, bias=b1_sb[sname][:, hh : hh + 1]
                    )

                def l2(sname=sname, hsb=hsb, stash=stash):
                    puv = ps_uv.tile([O, NB], f32, tag="puv", name="puv")
                    for hh in range(HH_T):
                        nc.tensor.matmul(
                            puv,
                            w2_sb[sname][:, hh, :],
                            hsb[hh],
                            start=(hh == 0),
                            stop=(hh == HH_T - 1),
                        )
                    usb = uvp.tile([O, NB], bf16, tag="uv", name="uv")
                    nc.vector.tensor_scalar_add(usb, puv, b2_sb[sname])
                    stash[sname] = usb

                after_mms(3, l2)

            def dot(b=b, stash=stash):
                prod = uvp.tile([O, NB], bf16, tag="prod", name="prod")
                nc.vector.tensor_mul(prod, stash["u"], stash["i"])
                pd = ps_d.tile([1, NB], f32, tag="pd", name="pd")
                nc.tensor.matmul(pd, ones_sb, prod, start=True, stop=True)
                s_blk = sp.tile([1, NB], f32, tag="sblk", name="s_blk")
                nc.scalar.activation(s_blk, pd, AF.Sigmoid)
                # Final block's output goes out on the (long-idle) sync
                # ring for the lightest completion path at kernel end.
                eng = nc.sync if b == NBLK - 1 else nc.gpsimd
                eng.dma_start(out[b * NB : (b + 1) * NB], s_blk)

            after_mms(6, dot)

        flush()

    nc.compile()
    return nc


def _pack_x(x):
    """[N_FULL, D] f32 -> per-core ([NBLK,128,KTB,NB] bf16, [NBLK,128,KT8,NB] fp8).

    packed_b[c][b, p, k, n] = x[c*NC_ROWS + b*NB + n, k*128 + p]        (k < KTB)
    packed_8[c][b, p, j, n] = x[c*NC_ROWS + b*NB + n, (KTB+j)*128 + p]  (fp8)
    """
    xf = np.asarray(x, dtype=np.float32)
    xb = xf[:, : KTB * 128].astype(ml_dtypes.bfloat16)
    x8 = xf[:, KTB * 128 :].astype(ml_dtypes.float8_e4m3fn)
    xb = xb.reshape(N_CORES, NBLK, NB, KTB, 128)
    x8 = x8.reshape(N_CORES, NBLK, NB, KT8, 128)
    return (
        [np.ascontiguousarray(xb[c].transpose(0, 3, 2, 1)) for c in range(N_CORES)],
        [np.ascontiguousarray(x8[c].transpose(0, 3, 2, 1)) for c in range(N_CORES)],
    )


def _pack_w1(w1):
    """[D, H] -> ([128, KTB, H] bf16, [128, KT8, H] fp8), scaled by W1SCALE."""
    wf = np.asarray(w1, dtype=np.float32) * np.float32(W1SCALE)
    wb = wf[: KTB * 128].astype(ml_dtypes.bfloat16)
    w8 = wf[KTB * 128 :].astype(ml_dtypes.float8_e4m3fn)
    return (
        np.ascontiguousarray(wb.reshape(KTB, 128, H).transpose(1, 0, 2)),
        np.ascontiguousarray(w8.reshape(KT8, 128, H).transpose(1, 0, 2)),
    )


def _pack_w2(w2):
    """[H, O] -> [128, HH_T, O] bf16, scaled by 1/W1SCALE."""
    wb = (np.asarray(w2, dtype=np.float32) / np.float32(W1SCALE)).astype(
        ml_dtypes.bfloat16
    )
    return np.ascontiguousarray(wb.reshape(HH_T, 128, O).transpose(1, 0, 2))


def _pack_cst(b1u, b1i, b2u, b2i):
    """[128, 6] f32: W1SCALE*b1u (2 cols), W1SCALE*b1i (2 cols), b2u, b2i."""
    cst = np.zeros((128, 6), dtype=np.float32)
    cst[:, 0:2] = np.float32(W1SCALE) * b1u.reshape(2, 128).T
    cst[:, 2:4] = np.float32(W1SCALE) * b1i.reshape(2, 128).T
    cst[: b2u.shape[0], 4] = b2u
    cst[: b2i.shape[0], 5] = b2i
    return cst


def _get_program():
    global _PROGRAM
    if _PROGRAM is None:
        _PROGRAM = _build_program()
    return _PROGRAM


def kernel(
    user_origin_emb,
    item_origin_emb,
    u_w1,
    u_b1,
    u_w2,
    u_b2,
    i_w1,
    i_b1,
    i_w2,
    i_b2,
):
    global LAST_RESULTS
    from concourse.bass_utils import run_bass_kernel_spmd

    xub, xu8 = _pack_x(user_origin_emb)
    xvb, xv8 = _pack_x(item_origin_emb)
    w1ub, w1u8 = _pack_w1(u_w1)
    w1ib, w1i8 = _pack_w1(i_w1)
    shared = {
        "w1ub": w1ub,
        "w1u8": w1u8,
        "w1ib": w1ib,
        "w1i8": w1i8,
        "w2u": _pack_w2(u_w2),
        "w2i": _pack_w2(i_w2),
        "cst": _pack_cst(
            np.asarray(u_b1, dtype=np.float32),
            np.asarray(i_b1, dtype=np.float32),
            np.asarray(u_b2, dtype=np.float32),
            np.asarray(i_b2, dtype=np.float32),
        ),
        "ones": np.ones((O, 1), dtype=ml_dtypes.bfloat16),
    }

    nc = _get_program()
    in_maps = [
        {"xub": xub[c], "xu8": xu8[c], "xvb": xvb[c], "xv8": xv8[c], **shared}
        for c in range(N_CORES)
    ]
    res = run_bass_kernel_spmd(nc, in_maps, core_ids=list(range(N_CORES)), trace=TRACE)
    LAST_RESULTS = res
    return np.concatenate([r["out"] for r in res.results], axis=0)


# revision 14
# speedup vs baseline: 1.1354x; 1.0348x over previous
"""Trainium2 Bass kernel for nn_Llama_head (paired two-tower MLP head).

Computes sigmoid(rowwise_dot(mlp_u(xu), mlp_i(xv))) for N=32768 rows,
data-parallel across 8 NeuronCores (N sharded, weights replicated).

Key structure:
  - Host pre-packs x transposed/tiled so the kernel has no on-chip
    transposes: bf16 for k-tiles [0,26), fp8-e4m3 for k-tiles [26,32).
    The fp8 k-tiles run as DoubleRow (double-pumped) matmuls; w1 is
    pre-scaled by 512 so its values sit in e4m3's normal range, with
    the inverse folded into b1 (x512, relu is positively homogeneous)
    and w2 (/512). Measured end-to-end error ~1.3e-2 vs the 2e-2 gate.
  - Layer 1: hT[h, n] += w1[dk, h].T @ xT[dk, n] into PSUM (26 bf16
    k-tiles + 3 DoubleRow fp8 pairs); ACT relu (+512*b1) -> bf16 h;
    layer 2 + row-dot matmuls are emitted a few L1 matmuls into the
    next tower's PE stream so the PE never waits on ACT/DVE.
  - DMA: sync HWDGE ring starts ~6us earlier than the others, so it
    carries the need-ordered startup (w1u/xu0/w1i/xv0, k-chunked) and
    even blocks; the scalar ring carries odd blocks; the gpsimd SWDGE
    ring carries tiny constants and per-block 2KB outputs.
"""

import os

import numpy as np
import ml_dtypes

# Problem shape (hardcoded per harness contract).
N_FULL = 32768
D = 4096
H = 256
O = 64
N_CORES = 8

NC_ROWS = N_FULL // N_CORES  # rows per core
NB = 512                     # rows per block
NBLK = NC_ROWS // NB
KT = D // 128                # layer-1 k-tiles
KT8 = 6                      # k-tiles computed in fp8 (DoubleRow pairs)
KTB = KT - KT8               # k-tiles computed in bf16
HH_T = H // 128              # layer-2 k-tiles (= layer-1 out tiles)
W1SCALE = 512.0              # w1 pre-scale so fp8 tiles avoid subnormals
TRACE = bool(int(os.environ.get("KERNEL_TRACE", "0")))

LAST_RESULTS = None  # BassKernelResults of the most recent run (for profiling)

_PROGRAM = None


def _build_program():
    from contextlib import ExitStack

    import concourse.mybir as mybir
    import concourse.tile as tile
    from concourse import bacc

    f32 = mybir.dt.float32
    bf16 = mybir.dt.bfloat16
    fp8 = mybir.dt.float8e4
    AF = mybir.ActivationFunctionType
    DR = mybir.MatmulPerfMode.DoubleRow

    nc = bacc.Bacc("TRN2")

    xb_d = {
        "u": nc.dram_tensor("xub", [NBLK, 128, KTB, NB], bf16, kind="ExternalInput"),
        "i": nc.dram_tensor("xvb", [NBLK, 128, KTB, NB], bf16, kind="ExternalInput"),
    }
    x8_d = {
        "u": nc.dram_tensor("xu8", [NBLK, 128, KT8, NB], fp8, kind="ExternalInput"),
        "i": nc.dram_tensor("xv8", [NBLK, 128, KT8, NB], fp8, kind="ExternalInput"),
    }
    w1b_d = {
        "u": nc.dram_tensor("w1ub", [128, KTB, H], bf16, kind="ExternalInput"),
        "i": nc.dram_tensor("w1ib", [128, KTB, H], bf16, kind="ExternalInput"),
    }
    w18_d = {
        "u": nc.dram_tensor("w1u8", [128, KT8, H], fp8, kind="ExternalInput"),
        "i": nc.dram_tensor("w1i8", [128, KT8, H], fp8, kind="ExternalInput"),
    }
    w2_d = {
        "u": nc.dram_tensor("w2u", [128, HH_T, O], bf16, kind="ExternalInput"),
        "i": nc.dram_tensor("w2i", [128, HH_T, O], bf16, kind="ExternalInput"),
    }
    cst_d = nc.dram_tensor("cst", [128, 6], f32, kind="ExternalInput")
    ones_d = nc.dram_tensor("ones", [O, 1], bf16, kind="ExternalInput")
    out = nc.dram_tensor("out", [NC_ROWS], f32, kind="ExternalOutput")

    with ExitStack() as ctx:
        tc = ctx.enter_context(tile.TileContext(nc))

        wpool = ctx.enter_context(tc.tile_pool(name="weights", bufs=1))
        xp = ctx.enter_context(tc.tile_pool(name="x", bufs=4))
        xp8 = ctx.enter_context(tc.tile_pool(name="x8", bufs=4))
        hp = ctx.enter_context(tc.tile_pool(name="h", bufs=4))
        uvp = ctx.enter_context(tc.tile_pool(name="uv", bufs=4))
        sp = ctx.enter_context(tc.tile_pool(name="sblk", bufs=2))
        ps_h = ctx.enter_context(tc.tile_pool(name="psh", bufs=4, space="PSUM"))
        ps_uv = ctx.enter_context(tc.tile_pool(name="psuv", bufs=2, space="PSUM"))
        ps_d = ctx.enter_context(tc.tile_pool(name="psd", bufs=2, space="PSUM"))

        w1b_sb = {}
        w18_sb = {}
        w2_sb = {}
        for s in ("u", "i"):
            w1b_sb[s] = wpool.tile([128, KTB, H], bf16, tag=f"w1b{s}", name=f"w1b{s}")
            w18_sb[s] = wpool.tile([128, KT8, H], fp8, tag=f"w18{s}", name=f"w18{s}")
            w2_sb[s] = wpool.tile([128, HH_T, O], bf16, tag=f"w2{s}", name=f"w2{s}")
        cst = wpool.tile([128, 6], f32, tag="cst", name="cst")
        ones_sb = wpool.tile([O, 1], bf16, tag="ones", name="ones")

        # Tiny constants + all fp8 x tiles + per-block outs ride the
        # gpsimd SWDGE ring (~30GB/s needed; the ring is otherwise idle).
        nc.gpsimd.dma_start(cst, cst_d[:])
        nc.gpsimd.dma_start(ones_sb, ones_d[:])

        b1_sb = {"u": cst[:, 0:2], "i": cst[:, 2:4]}
        b2_sb = {"u": cst[:O, 4:5], "i": cst[:O, 5:6]}

        nat_b = {}
        nat_8 = {}
        for s in ("u", "i"):
            nat_b[s] = xp.tile([128, KTB, NB], bf16, tag="x", name="x")
            nat_8[s] = xp8.tile([128, KT8, NB], fp8, tag="x8", name="x8")

        # Startup: all of block 0 (weights + both towers) on the sync
        # ring — it starts ~6us before the others — k-chunked in exact
        # consumption order. fp8 parts on gpsimd.
        for s in ("u", "i"):
            nc.sync.dma_start(w1b_sb[s][:, 0:4, :], w1b_d[s][:, 0:4, :])
            nc.sync.dma_start(nat_b[s][:, 0:4, :], xb_d[s][0][:, 0:4, :])
            nc.sync.dma_start(w1b_sb[s][:, 4:13, :], w1b_d[s][:, 4:13, :])
            nc.sync.dma_start(nat_b[s][:, 4:13, :], xb_d[s][0][:, 4:13, :])
            nc.sync.dma_start(w1b_sb[s][:, 13:KTB, :], w1b_d[s][:, 13:KTB, :])
            nc.sync.dma_start(nat_b[s][:, 13:KTB, :], xb_d[s][0][:, 13:KTB, :])
            nc.gpsimd.dma_start(w18_sb[s], w18_d[s][:])
            nc.gpsimd.dma_start(nat_8[s], x8_d[s][0])
        nc.gpsimd.dma_start(w2_sb["u"], w2_d["u"][:])
        nc.gpsimd.dma_start(w2_sb["i"], w2_d["i"][:])

        # --- deferred PE emission machinery (see module docstring).
        pending = []

        def after_mms(n, fn):
            pending.append([n, fn])

        def tick():
            due = [it for it in pending if it[0] <= 1]
            for it in due:
                pending.remove(it)
                it[1]()
            for it in pending:
                it[0] -= 1

        def flush():
            while pending:
                pending.pop(0)[1]()

        for b in range(NBLK):
            stash = {}
            for sname in ("u", "i"):
                if b == 0:
                    nat = nat_b[sname]
                    nat8 = nat_8[sname]
                else:
                    # Even blocks on the sync ring, odd on the scalar
                    # ring (block-granular ping-pong keeps each ring's
                    # completion order equal to consumption order).
                    # Odd-block tiles are halved: the scalar ring runs
                    # closer to the PE's need-times, and half-tile waits
                    # let the PE start on the first half.
                    eng = nc.sync if b % 2 == 0 else nc.scalar
                    nat = xp.tile([128, KTB, NB], bf16, tag="x", name="x")
                    eng.dma_start(nat[:, 0:13, :], xb_d[sname][b][:, 0:13, :])
                    eng.dma_start(nat[:, 13:, :], xb_d[sname][b][:, 13:, :])
                    nat8 = xp8.tile([128, KT8, NB], fp8, tag="x8", name="x8")
                    nc.gpsimd.dma_start(nat8, x8_d[sname][b])

                ph = [ps_h.tile([128, NB], f32, tag="ph", name="ph") for _ in range(HH_T)]
                hsb = [hp.tile([128, NB], bf16, tag="h", name="h") for _ in range(HH_T)]
                for hh in range(HH_T):
                    hs = slice(hh * 128, (hh + 1) * 128)
                    for k in range(KTB):
                        nc.tensor.matmul(
                            ph[hh],
                            w1b_sb[sname][:, k, hs],
                            nat[:, k, :],
                            start=(k == 0),
                            stop=False,
                        )
                        tick()
                    for j in range(KT8 // 2):
                        nc.tensor.matmul(
                            ph[hh],
                            w18_sb[sname][:, 2 * j : 2 * j + 2, hs],
                            nat8[:, 2 * j : 2 * j + 2, :],
                            start=False,
                            stop=(j == KT8 // 2 - 1),
                            perf_mode=DR,
                        )
                        tick()
                    nc.scalar.activation(
                        hsb[hh], ph[hh], AF.Relu, bias=b1_sb[sname][:, hh : hh + 1]
                    )

                cell = {}

                def l2a(sname=sname, hsb=hsb, cell=cell):
                    puv = ps_uv.tile([O, NB], f32, tag="puv", name="puv")
                    cell["puv"] = puv
                    nc.tensor.matmul(
                        puv, w2_sb[sname][:, 0, :], hsb[0], start=True, stop=False
                    )

                def l2b(sname=sname, hsb=hsb, cell=cell, stash=stash):
                    puv = cell["puv"]
                    nc.tensor.matmul(
                        puv, w2_sb[sname][:, 1, :], hsb[1], start=False, stop=True
                    )
                    usb = uvp.tile([O, NB], bf16, tag="uv", name="uv")
                    nc.vector.tensor_scalar_add(usb, puv, b2_sb[sname])
                    stash[sname] = usb

                after_mms(3, l2a)
                after_mms(6, l2b)

            def dot(b=b, stash=stash):
                prod = uvp.tile([O, NB], bf16, tag="prod", name="prod")
                nc.vector.tensor_mul(prod, stash["u"], stash["i"])
                pd = ps_d.tile([1, NB], f32, tag="pd", name="pd")
                nc.tensor.matmul(pd, ones_sb, prod, start=True, stop=True)
                s_blk = sp.tile([1, NB], f32, tag="sblk", name="s_blk")
                nc.scalar.activation(s_blk, pd, AF.Sigmoid)
                # Final block's output goes out on the (long-idle) sync
                # ring for the lightest completion path at kernel end.
                eng = nc.sync if b == NBLK - 1 else nc.gpsimd
                eng.dma_start(out[b * NB : (b + 1) * NB], s_blk)

            after_mms(9, dot)

        flush()

    nc.compile()
    return nc


def _pack_x(x):
    """[N_FULL, D] f32 -> per-core ([NBLK,128,KTB,NB] bf16, [NBLK,128,KT8,NB] fp8).

    packed_b[c][b, p, k, n] = x[c*NC_ROWS + b*NB + n, k*128 + p]        (k < KTB)
    packed_8[c][b, p, j, n] = x[c*NC_ROWS + b*NB + n, (KTB+j)*128 + p]  (fp8)
    """
    xf = np.asarray(x, dtype=np.float32)
    xb = xf[:, : KTB * 128].astype(ml_dtypes.bfloat16)
    x8 = xf[:, KTB * 128 :].astype(ml_dtypes.float8_e4m3fn)
    xb = xb.reshape(N_CORES, NBLK, NB, KTB, 128)
    x8 = x8.reshape(N_CORES, NBLK, NB, KT8, 128)
    return (
        [np.ascontiguousarray(xb[c].transpose(0, 3, 2, 1)) for c in range(N_CORES)],
        [np.ascontiguousarray(x8[c].transpose(0, 3, 2, 1)) for c in range(N_CORES)],
    )


def _pack_w1(w1):
    """[D, H] -> ([128, KTB, H] bf16, [128, KT8, H] fp8), scaled by W1SCALE."""
    wf = np.asarray(w1, dtype=np.float32) * np.float32(W1SCALE)
    wb = wf[: KTB * 128].astype(ml_dtypes.bfloat16)
    w8 = wf[KTB * 128 :].astype(ml_dtypes.float8_e4m3fn)
    return (
        np.ascontiguousarray(wb.reshape(KTB, 128, H).transpose(1, 0, 2)),
        np.ascontiguousarray(w8.reshape(KT8, 128, H).transpose(1, 0, 2)),
    )


def _pack_w2(w2):
    """[H, O] -> [128, HH_T, O] bf16, scaled by 1/W1SCALE."""
    wb = (np.asarray(w2, dtype=np.float32) / np.float32(W1SCALE)).astype(
        ml_dtypes.bfloat16
    )
    return np.ascontiguousarray(wb.reshape(HH_T, 128, O).transpose(1, 0, 2))


def _pack_cst(b1u, b1i, b2u, b2i):
    """[128, 6] f32: W1SCALE*b1u (2 cols), W1SCALE*b1i (2 cols), b2u, b2i."""
    cst = np.zeros((128, 6), dtype=np.float32)
    cst[:, 0:2] = np.float32(W1SCALE) * b1u.reshape(2, 128).T
    cst[:, 2:4] = np.float32(W1SCALE) * b1i.reshape(2, 128).T
    cst[: b2u.shape[0], 4] = b2u
    cst[: b2i.shape[0], 5] = b2i
    return cst


def _get_program():
    global _PROGRAM
    if _PROGRAM is None:
        _PROGRAM = _build_program()
    return _PROGRAM


def kernel(
    user_origin_emb,
    item_origin_emb,
    u_w1,
    u_b1,
    u_w2,
    u_b2,
    i_w1,
    i_b1,
    i_w2,
    i_b2,
):
    global LAST_RESULTS
    from concourse.bass_utils import run_bass_kernel_spmd

    xub, xu8 = _pack_x(user_origin_emb)
    xvb, xv8 = _pack_x(item_origin_emb)
    w1ub, w1u8 = _pack_w1(u_w1)
    w1ib, w1i8 = _pack_w1(i_w1)
    shared = {
        "w1ub": w1ub,
        "w1u8": w1u8,
        "w1ib": w1ib,
        "w1i8": w1i8,
        "w2u": _pack_w2(u_w2),
        "w2i": _pack_w2(i_w2),
        "cst": _pack_cst(
            np.asarray(u_b1, dtype=np.float32),
            np.asarray(i_b1, dtype=np.float32),
            np.asarray(u_b2, dtype=np.float32),
            np.asarray(i_b2, dtype=np.float32),
        ),
        "ones": np.ones((O, 1), dtype=ml_dtypes.bfloat16),
    }

    nc = _get_program()
    in_maps = [
        {"xub": xub[c], "xu8": xu8[c], "xvb": xvb[c], "xv8": xv8[c], **shared}
        for c in range(N_CORES)
    ]
    res = run_bass_kernel_spmd(nc, in_maps, core_ids=list(range(N_CORES)), trace=TRACE)
    LAST_RESULTS = res
    return np.concatenate([r["out"] for r in res.results], axis=0)


# revision 15
# speedup vs baseline: 1.1528x; 1.0153x over previous
"""Trainium2 Bass kernel for nn_Llama_head (paired two-tower MLP head).

Computes sigmoid(rowwise_dot(mlp_u(xu), mlp_i(xv))) for N=32768 rows,
data-parallel across 8 NeuronCores (N sharded, weights replicated).

Key structure:
  - Host pre-packs x transposed/tiled so the kernel has no on-chip
    transposes: bf16 for k-tiles [0,26), fp8-e4m3 for k-tiles [26,32).
    The fp8 k-tiles run as DoubleRow (double-pumped) matmuls; w1 is
    pre-scaled by 512 so its values sit in e4m3's normal range, with
    the inverse folded into b1 (x512, relu is positively homogeneous)
    and w2 (/512). Measured end-to-end error ~1.3e-2 vs the 2e-2 gate.
  - Layer 1: hT[h, n] += w1[dk, h].T @ xT[dk, n] into PSUM (26 bf16
    k-tiles + 3 DoubleRow fp8 pairs); ACT relu (+512*b1) -> bf16 h;
    layer 2 + row-dot matmuls are emitted a few L1 matmuls into the
    next tower's PE stream so the PE never waits on ACT/DVE.
  - DMA: sync HWDGE ring starts ~6us earlier than the others, so it
    carries the need-ordered startup (w1u/xu0/w1i/xv0, k-chunked) and
    even blocks; the scalar ring carries odd blocks; the gpsimd SWDGE
    ring carries tiny constants and per-block 2KB outputs.
"""

import os

import numpy as np
import ml_dtypes

# Problem shape (hardcoded per harness contract).
N_FULL = 32768
D = 4096
H = 256
O = 64
N_CORES = 8

NC_ROWS = N_FULL // N_CORES  # rows per core
NB = 512                     # rows per block
NBLK = NC_ROWS // NB
KT = D // 128                # layer-1 k-tiles
KT8 = 8                      # k-tiles computed in fp8 (DoubleRow pairs)
KTB = KT - KT8               # k-tiles computed in bf16
HH_T = H // 128              # layer-2 k-tiles (= layer-1 out tiles)
W1SCALE = 512.0              # w1 pre-scale so fp8 tiles avoid subnormals
TRACE = bool(int(os.environ.get("KERNEL_TRACE", "0")))

LAST_RESULTS = None  # BassKernelResults of the most recent run (for profiling)

_PROGRAM = None


def _build_program():
    from contextlib import ExitStack

    import concourse.mybir as mybir
    import concourse.tile as tile
    from concourse import bacc

    f32 = mybir.dt.float32
    bf16 = mybir.dt.bfloat16
    fp8 = mybir.dt.float8e4
    AF = mybir.ActivationFunctionType
    DR = mybir.MatmulPerfMode.DoubleRow

    nc = bacc.Bacc("TRN2")

    xb_d = {
        "u": nc.dram_tensor("xub", [NBLK, 128, KTB, NB], bf16, kind="ExternalInput"),
        "i": nc.dram_tensor("xvb", [NBLK, 128, KTB, NB], bf16, kind="ExternalInput"),
    }
    x8_d = {
        "u": nc.dram_tensor("xu8", [NBLK, 128, KT8, NB], fp8, kind="ExternalInput"),
        "i": nc.dram_tensor("xv8", [NBLK, 128, KT8, NB], fp8, kind="ExternalInput"),
    }
    w1b_d = {
        "u": nc.dram_tensor("w1ub", [128, HH_T, KTB, 128], bf16, kind="ExternalInput"),
        "i": nc.dram_tensor("w1ib", [128, HH_T, KTB, 128], bf16, kind="ExternalInput"),
    }
    w18_d = {
        "u": nc.dram_tensor("w1u8", [128, HH_T, KT8, 128], fp8, kind="ExternalInput"),
        "i": nc.dram_tensor("w1i8", [128, HH_T, KT8, 128], fp8, kind="ExternalInput"),
    }
    w2_d = {
        "u": nc.dram_tensor("w2u", [128, HH_T, O], bf16, kind="ExternalInput"),
        "i": nc.dram_tensor("w2i", [128, HH_T, O], bf16, kind="ExternalInput"),
    }
    cst_d = nc.dram_tensor("cst", [128, 6], f32, kind="ExternalInput")
    ones_d = nc.dram_tensor("ones", [O, 1], bf16, kind="ExternalInput")
    out = nc.dram_tensor("out", [NC_ROWS], f32, kind="ExternalOutput")

    with ExitStack() as ctx:
        tc = ctx.enter_context(tile.TileContext(nc))

        wpool = ctx.enter_context(tc.tile_pool(name="weights", bufs=1))
        xp = ctx.enter_context(tc.tile_pool(name="x", bufs=4))
        xp8 = ctx.enter_context(tc.tile_pool(name="x8", bufs=4))
        hp = ctx.enter_context(tc.tile_pool(name="h", bufs=4))
        uvp = ctx.enter_context(tc.tile_pool(name="uv", bufs=4))
        sp = ctx.enter_context(tc.tile_pool(name="sblk", bufs=2))
        ps_h = ctx.enter_context(tc.tile_pool(name="psh", bufs=4, space="PSUM"))
        ps_uv = ctx.enter_context(tc.tile_pool(name="psuv", bufs=2, space="PSUM"))
        ps_d = ctx.enter_context(tc.tile_pool(name="psd", bufs=2, space="PSUM"))

        w1b_sb = {}
        w18_sb = {}
        w2_sb = {}
        for s in ("u", "i"):
            w1b_sb[s] = wpool.tile([128, HH_T, KTB, 128], bf16, tag=f"w1b{s}", name=f"w1b{s}")
            w18_sb[s] = wpool.tile([128, HH_T, KT8, 128], fp8, tag=f"w18{s}", name=f"w18{s}")
            w2_sb[s] = wpool.tile([128, HH_T, O], bf16, tag=f"w2{s}", name=f"w2{s}")
        cst = wpool.tile([128, 6], f32, tag="cst", name="cst")
        ones_sb = wpool.tile([O, 1], bf16, tag="ones", name="ones")

        # Tiny constants + all fp8 x tiles + per-block outs ride the
        # gpsimd SWDGE ring (~30GB/s needed; the ring is otherwise idle).
        nc.gpsimd.dma_start(cst, cst_d[:])
        nc.gpsimd.dma_start(ones_sb, ones_d[:])

        b1_sb = {"u": cst[:, 0:2], "i": cst[:, 2:4]}
        b2_sb = {"u": cst[:O, 4:5], "i": cst[:O, 5:6]}

        nat_b = {}
        nat_8 = {}
        for s in ("u", "i"):
            nat_b[s] = xp.tile([128, KTB, NB], bf16, tag="x", name="x")
            nat_8[s] = xp8.tile([128, KT8, NB], fp8, tag="x8", name="x8")

        # Startup: all of block 0 (weights + both towers) on the sync
        # ring — it starts ~6us before the others — k-chunked in exact
        # consumption order. fp8 parts on gpsimd.
        for s in ("u", "i"):
            nc.sync.dma_start(w1b_sb[s][:, 0, 0:4, :], w1b_d[s][:, 0, 0:4, :])
            nc.sync.dma_start(nat_b[s][:, 0:4, :], xb_d[s][0][:, 0:4, :])
            nc.sync.dma_start(w1b_sb[s][:, 0, 4:12, :], w1b_d[s][:, 0, 4:12, :])
            nc.sync.dma_start(nat_b[s][:, 4:12, :], xb_d[s][0][:, 4:12, :])
            nc.sync.dma_start(w1b_sb[s][:, 0, 12:KTB, :], w1b_d[s][:, 0, 12:KTB, :])
            nc.sync.dma_start(nat_b[s][:, 12:KTB, :], xb_d[s][0][:, 12:KTB, :])
            nc.sync.dma_start(w1b_sb[s][:, 1], w1b_d[s][:, 1])
            nc.gpsimd.dma_start(w18_sb[s], w18_d[s][:])
            nc.gpsimd.dma_start(nat_8[s], x8_d[s][0])
        nc.gpsimd.dma_start(w2_sb["u"], w2_d["u"][:])
        nc.gpsimd.dma_start(w2_sb["i"], w2_d["i"][:])

        # --- deferred PE emission machinery (see module docstring).
        pending = []

        def after_mms(n, fn):
            pending.append([n, fn])

        def tick():
            due = [it for it in pending if it[0] <= 1]
            for it in due:
                pending.remove(it)
                it[1]()
            for it in pending:
                it[0] -= 1

        def flush():
            while pending:
                pending.pop(0)[1]()

        for b in range(NBLK):
            stash = {}
            for sname in ("u", "i"):
                if b == 0:
                    nat = nat_b[sname]
                    nat8 = nat_8[sname]
                else:
                    # Even blocks on the sync ring, odd on the scalar
                    # ring (block-granular ping-pong keeps each ring's
                    # completion order equal to consumption order).
                    # Odd-block tiles are halved: the scalar ring runs
                    # closer to the PE's need-times, and half-tile waits
                    # let the PE start on the first half.
                    eng = nc.sync if b % 2 == 0 else nc.scalar
                    nat = xp.tile([128, KTB, NB], bf16, tag="x", name="x")
                    eng.dma_start(nat[:, 0:12, :], xb_d[sname][b][:, 0:12, :])
                    eng.dma_start(nat[:, 12:, :], xb_d[sname][b][:, 12:, :])
                    nat8 = xp8.tile([128, KT8, NB], fp8, tag="x8", name="x8")
                    nc.gpsimd.dma_start(nat8, x8_d[sname][b])

                ph = [ps_h.tile([128, NB], f32, tag="ph", name="ph") for _ in range(HH_T)]
                hsb = [hp.tile([128, NB], bf16, tag="h", name="h") for _ in range(HH_T)]
                for hh in range(HH_T):
                    for k in range(KTB):
                        nc.tensor.matmul(
                            ph[hh],
                            w1b_sb[sname][:, hh, k, :],
                            nat[:, k, :],
                            start=(k == 0),
                            stop=False,
                        )
                        tick()
                    for j in range(KT8 // 2):
                        nc.tensor.matmul(
                            ph[hh],
                            w18_sb[sname][:, hh, 2 * j : 2 * j + 2, :],
                            nat8[:, 2 * j : 2 * j + 2, :],
                            start=False,
                            stop=(j == KT8 // 2 - 1),
                            perf_mode=DR,
                        )
                        tick()
                    nc.scalar.activation(
                        hsb[hh], ph[hh], AF.Relu, bias=b1_sb[sname][:, hh : hh + 1]
                    )

                cell = {}

                def l2a(sname=sname, hsb=hsb, cell=cell):
                    puv = ps_uv.tile([O, NB], f32, tag="puv", name="puv")
                    cell["puv"] = puv
                    nc.tensor.matmul(
                        puv, w2_sb[sname][:, 0, :], hsb[0], start=True, stop=False
                    )

                def l2b(sname=sname, hsb=hsb, cell=cell, stash=stash):
                    puv = cell["puv"]
                    nc.tensor.matmul(
                        puv, w2_sb[sname][:, 1, :], hsb[1], start=False, stop=True
                    )
                    usb = uvp.tile([O, NB], bf16, tag="uv", name="uv")
                    nc.vector.tensor_scalar_add(usb, puv, b2_sb[sname])
                    stash[sname] = usb

                after_mms(3, l2a)
                after_mms(6, l2b)

            def dot(b=b, stash=stash):
                prod = uvp.tile([O, NB], bf16, tag="prod", name="prod")
                nc.vector.tensor_mul(prod, stash["u"], stash["i"])
                pd = ps_d.tile([1, NB], f32, tag="pd", name="pd")
                nc.tensor.matmul(pd, ones_sb, prod, start=True, stop=True)
                s_blk = sp.tile([1, NB], f32, tag="sblk", name="s_blk")
                nc.scalar.activation(s_blk, pd, AF.Sigmoid)
                # Final block's output goes out on the (long-idle) sync
                # ring for the lightest completion path at kernel end.
                eng = nc.sync if b == NBLK - 1 else nc.gpsimd
                eng.dma_start(out[b * NB : (b + 1) * NB], s_blk)

            after_mms(9, dot)

        flush()

    nc.compile()
    return nc


def _pack_x(x):
    """[N_FULL, D] f32 -> per-core ([NBLK,128,KTB,NB] bf16, [NBLK,128,KT8,NB] fp8).

    packed_b[c][b, p, k, n] = x[c*NC_ROWS + b*NB + n, k*128 + p]        (k < KTB)
    packed_8[c][b, p, j, n] = x[c*NC_ROWS + b*NB + n, (KTB+j)*128 + p]  (fp8)
    """
    xf = np.asarray(x, dtype=np.float32)
    xb = xf[:, : KTB * 128].astype(ml_dtypes.bfloat16)
    x8 = xf[:, KTB * 128 :].astype(ml_dtypes.float8_e4m3fn)
    xb = xb.reshape(N_CORES, NBLK, NB, KTB, 128)
    x8 = x8.reshape(N_CORES, NBLK, NB, KT8, 128)
    return (
        [np.ascontiguousarray(xb[c].transpose(0, 3, 2, 1)) for c in range(N_CORES)],
        [np.ascontiguousarray(x8[c].transpose(0, 3, 2, 1)) for c in range(N_CORES)],
    )


def _pack_w1(w1):
    """[D, H] -> hh-major ([128, HH_T, KTB, 128] bf16, [128, HH_T, KT8, 128] fp8).

    element (p, a, k, m) = W1SCALE * w1[k*128 + p, a*128 + m]
    """
    wf = np.asarray(w1, dtype=np.float32) * np.float32(W1SCALE)
    wb = wf[: KTB * 128].astype(ml_dtypes.bfloat16)
    w8 = wf[KTB * 128 :].astype(ml_dtypes.float8_e4m3fn)
    wb = wb.reshape(KTB, 128, HH_T, 128).transpose(1, 2, 0, 3)
    w8 = w8.reshape(KT8, 128, HH_T, 128).transpose(1, 2, 0, 3)
    return np.ascontiguousarray(wb), np.ascontiguousarray(w8)


def _pack_w2(w2):
    """[H, O] -> [128, HH_T, O] bf16, scaled by 1/W1SCALE."""
    wb = (np.asarray(w2, dtype=np.float32) / np.float32(W1SCALE)).astype(
        ml_dtypes.bfloat16
    )
    return np.ascontiguousarray(wb.reshape(HH_T, 128, O).transpose(1, 0, 2))


def _pack_cst(b1u, b1i, b2u, b2i):
    """[128, 6] f32: W1SCALE*b1u (2 cols), W1SCALE*b1i (2 cols), b2u, b2i."""
    cst = np.zeros((128, 6), dtype=np.float32)
    cst[:, 0:2] = np.float32(W1SCALE) * b1u.reshape(2, 128).T
    cst[:, 2:4] = np.float32(W1SCALE) * b1i.reshape(2, 128).T
    cst[: b2u.shape[0], 4] = b2u
    cst[: b2i.shape[0], 5] = b2i
    return cst


def _get_program():
    global _PROGRAM
    if _PROGRAM is None:
        _PROGRAM = _build_program()
    return _PROGRAM


def kernel(
    user_origin_emb,
    item_origin_emb,
    u_w1,
    u_b1,
    u_w2,
    u_b2,
    i_w1,
    i_b1,
    i_w2,
    i_b2,
):
    global LAST_RESULTS
    from concourse.bass_utils import run_bass_kernel_spmd

    xub, xu8 = _pack_x(user_origin_emb)
    xvb, xv8 = _pack_x(item_origin_emb)
    w1ub, w1u8 = _pack_w1(u_w1)
    w1ib, w1i8 = _pack_w1(i_w1)
    shared = {
        "w1ub": w1ub,
        "w1u8": w1u8,
        "w1ib": w1ib,
        "w1i8": w1i8,
        "w2u": _pack_w2(u_w2),
        "w2i": _pack_w2(i_w2),
        "cst": _pack_cst(
            np.asarray(u_b1, dtype=np.float32),
            np.asarray(i_b1, dtype=np.float32),
            np.asarray(u_b2, dtype=np.float32),
            np.asarray(i_b2, dtype=np.float32),
        ),
        "ones": np.ones((O, 1), dtype=ml_dtypes.bfloat16),
    }

    nc = _get_program()
    in_maps = [
        {"xub": xub[c], "xu8": xu8[c], "xvb": xvb[c], "xv8": xv8[c], **shared}
        for c in range(N_CORES)
    ]
    res = run_bass_kernel_spmd(nc, in_maps, core_ids=list(range(N_CORES)), trace=TRACE)
    LAST_RESULTS = res
    return np.concatenate([r["out"] for r in res.results], axis=0)


# revision 16
# speedup vs baseline: 1.1791x; 1.0228x over previous
"""Trainium2 Bass kernel for nn_Llama_head (paired two-tower MLP head).

Computes sigmoid(rowwise_dot(mlp_u(xu), mlp_i(xv))) for N=32768 rows,
data-parallel across 8 NeuronCores (N sharded, weights replicated).

Key structure:
  - Host pre-packs x transposed/tiled so the kernel has no on-chip
    transposes: bf16 for k-tiles [0,26), fp8-e4m3 for k-tiles [26,32).
    The fp8 k-tiles run as DoubleRow (double-pumped) matmuls; w1 is
    pre-scaled by 512 so its values sit in e4m3's normal range, with
    the inverse folded into b1 (x512, relu is positively homogeneous)
    and w2 (/512). Measured end-to-end error ~1.3e-2 vs the 2e-2 gate.
  - Layer 1: hT[h, n] += w1[dk, h].T @ xT[dk, n] into PSUM (26 bf16
    k-tiles + 3 DoubleRow fp8 pairs); ACT relu (+512*b1) -> bf16 h;
    layer 2 + row-dot matmuls are emitted a few L1 matmuls into the
    next tower's PE stream so the PE never waits on ACT/DVE.
  - DMA: sync HWDGE ring starts ~6us earlier than the others, so it
    carries the need-ordered startup (w1u/xu0/w1i/xv0, k-chunked) and
    even blocks; the scalar ring carries odd blocks; the gpsimd SWDGE
    ring carries tiny constants and per-block 2KB outputs.
"""

import os

import numpy as np
import ml_dtypes

# Problem shape (hardcoded per harness contract).
N_FULL = 32768
D = 4096
H = 256
O = 64
N_CORES = 8

NC_ROWS = N_FULL // N_CORES  # rows per core
NB = 512                     # rows per block
NBLK = NC_ROWS // NB
KT = D // 128                # layer-1 k-tiles
KT8 = 8                      # k-tiles computed in fp8 (DoubleRow pairs)
KTB = KT - KT8               # k-tiles computed in bf16
HH_T = H // 128              # layer-2 k-tiles (= layer-1 out tiles)
W1SCALE = 512.0              # w1 pre-scale so fp8 tiles avoid subnormals
TRACE = bool(int(os.environ.get("KERNEL_TRACE", "0")))

LAST_RESULTS = None  # BassKernelResults of the most recent run (for profiling)

_PROGRAM = None


def _build_program():
    from contextlib import ExitStack

    import concourse.mybir as mybir
    import concourse.tile as tile
    from concourse import bacc

    f32 = mybir.dt.float32
    bf16 = mybir.dt.bfloat16
    fp8 = mybir.dt.float8e4
    AF = mybir.ActivationFunctionType
    DR = mybir.MatmulPerfMode.DoubleRow

    nc = bacc.Bacc("TRN2")

    xb_d = {
        "u": nc.dram_tensor("xub", [NBLK, 128, KTB, NB], bf16, kind="ExternalInput"),
        "i": nc.dram_tensor("xvb", [NBLK, 128, KTB, NB], bf16, kind="ExternalInput"),
    }
    x8_d = {
        "u": nc.dram_tensor("xu8", [NBLK, 128, KT8, NB], fp8, kind="ExternalInput"),
        "i": nc.dram_tensor("xv8", [NBLK, 128, KT8, NB], fp8, kind="ExternalInput"),
    }
    w1b_d = {
        "u": nc.dram_tensor("w1ub", [128, HH_T, KTB, 128], bf16, kind="ExternalInput"),
        "i": nc.dram_tensor("w1ib", [128, HH_T, KTB, 128], bf16, kind="ExternalInput"),
    }
    w18_d = {
        "u": nc.dram_tensor("w1u8", [128, HH_T, KT8, 128], fp8, kind="ExternalInput"),
        "i": nc.dram_tensor("w1i8", [128, HH_T, KT8, 128], fp8, kind="ExternalInput"),
    }
    w2_d = {
        "u": nc.dram_tensor("w2u", [128, HH_T, O], bf16, kind="ExternalInput"),
        "i": nc.dram_tensor("w2i", [128, HH_T, O], bf16, kind="ExternalInput"),
    }
    cst_d = nc.dram_tensor("cst", [128, 6], f32, kind="ExternalInput")
    ones_d = nc.dram_tensor("ones", [O, 1], bf16, kind="ExternalInput")
    out = nc.dram_tensor("out", [NC_ROWS], f32, kind="ExternalOutput")

    with ExitStack() as ctx:
        tc = ctx.enter_context(tile.TileContext(nc))

        wpool = ctx.enter_context(tc.tile_pool(name="weights", bufs=1))
        xp = ctx.enter_context(tc.tile_pool(name="x", bufs=4))
        xp8 = ctx.enter_context(tc.tile_pool(name="x8", bufs=4))
        hp = ctx.enter_context(tc.tile_pool(name="h", bufs=4))
        uvp = ctx.enter_context(tc.tile_pool(name="uv", bufs=4))
        sp = ctx.enter_context(tc.tile_pool(name="sblk", bufs=2))
        ps_h = ctx.enter_context(tc.tile_pool(name="psh", bufs=4, space="PSUM"))
        ps_uv = ctx.enter_context(tc.tile_pool(name="psuv", bufs=2, space="PSUM"))
        ps_d = ctx.enter_context(tc.tile_pool(name="psd", bufs=2, space="PSUM"))

        w1b_sb = {}
        w18_sb = {}
        w2_sb = {}
        for s in ("u", "i"):
            w1b_sb[s] = wpool.tile([128, HH_T, KTB, 128], bf16, tag=f"w1b{s}", name=f"w1b{s}")
            w18_sb[s] = wpool.tile([128, HH_T, KT8, 128], fp8, tag=f"w18{s}", name=f"w18{s}")
            w2_sb[s] = wpool.tile([128, HH_T, O], bf16, tag=f"w2{s}", name=f"w2{s}")
        cst = wpool.tile([128, 6], f32, tag="cst", name="cst")
        ones_sb = wpool.tile([O, 1], bf16, tag="ones", name="ones")

        # Tiny constants + all fp8 x tiles + per-block outs ride the
        # gpsimd SWDGE ring (~30GB/s needed; the ring is otherwise idle).
        nc.gpsimd.dma_start(cst, cst_d[:])
        nc.gpsimd.dma_start(ones_sb, ones_d[:])

        b1_sb = {"u": cst[:, 0:2], "i": cst[:, 2:4]}
        b2_sb = {"u": cst[:O, 4:5], "i": cst[:O, 5:6]}

        nat_b = {}
        nat_8 = {}
        for s in ("u", "i"):
            nat_b[s] = xp.tile([128, KTB, NB], bf16, tag="x", name="x")
            nat_8[s] = xp8.tile([128, KT8, NB], fp8, tag="x8", name="x8")

        # Startup: block-0 data striped across BOTH HWDGE rings in
        # k-chunks matching consumption order — each ring's queue is a
        # subsequence of the consumption sequence, so per-ring FIFO
        # completion order stays need-ordered and stalls stay short
        # (under the ~3.4us HAM re-throttle window). fp8 on gpsimd.
        nc.gpsimd.dma_start(w18_sb["u"], w18_d["u"][:])
        nc.gpsimd.dma_start(nat_8["u"], x8_d["u"][0])
        nc.gpsimd.dma_start(w18_sb["i"], w18_d["i"][:])
        nc.gpsimd.dma_start(nat_8["i"], x8_d["i"][0])
        nc.gpsimd.dma_start(w2_sb["u"], w2_d["u"][:])
        nc.gpsimd.dma_start(w2_sb["i"], w2_d["i"][:])

        nc.sync.dma_start(w1b_sb["u"][:, 0, 0:4, :], w1b_d["u"][:, 0, 0:4, :])
        nc.sync.dma_start(nat_b["u"][:, 0:4, :], xb_d["u"][0][:, 0:4, :])
        nc.scalar.dma_start(nat_b["u"][:, 4:8, :], xb_d["u"][0][:, 4:8, :])
        nc.sync.dma_start(w1b_sb["u"][:, 0, 4:KTB, :], w1b_d["u"][:, 0, 4:KTB, :])
        nc.scalar.dma_start(w1b_sb["u"][:, 1], w1b_d["u"][:, 1])
        nc.sync.dma_start(nat_b["u"][:, 8:16, :], xb_d["u"][0][:, 8:16, :])
        nc.scalar.dma_start(nat_b["u"][:, 16:KTB, :], xb_d["u"][0][:, 16:KTB, :])
        nc.sync.dma_start(w1b_sb["i"][:, 0], w1b_d["i"][:, 0])
        nc.sync.dma_start(nat_b["i"][:, 0:8, :], xb_d["i"][0][:, 0:8, :])
        nc.scalar.dma_start(nat_b["i"][:, 8:16, :], xb_d["i"][0][:, 8:16, :])
        nc.scalar.dma_start(w1b_sb["i"][:, 1], w1b_d["i"][:, 1])
        nc.sync.dma_start(nat_b["i"][:, 16:KTB, :], xb_d["i"][0][:, 16:KTB, :])

        # --- deferred PE emission machinery (see module docstring).
        pending = []

        def after_mms(n, fn):
            pending.append([n, fn])

        def tick():
            due = [it for it in pending if it[0] <= 1]
            for it in due:
                pending.remove(it)
                it[1]()
            for it in pending:
                it[0] -= 1

        def flush():
            while pending:
                pending.pop(0)[1]()

        for b in range(NBLK):
            stash = {}
            for sname in ("u", "i"):
                if b == 0:
                    nat = nat_b[sname]
                    nat8 = nat_8[sname]
                else:
                    nat = xp.tile([128, KTB, NB], bf16, tag="x", name="x")
                    if b == 1:
                        # Block 1 halves striped across both rings.
                        nc.sync.dma_start(nat[:, 0:12, :], xb_d[sname][b][:, 0:12, :])
                        nc.scalar.dma_start(nat[:, 12:, :], xb_d[sname][b][:, 12:, :])
                    else:
                        # Whole tiles, even blocks on sync, odd on scalar.
                        eng = nc.sync if b % 2 == 0 else nc.scalar
                        eng.dma_start(nat, xb_d[sname][b])
                    nat8 = xp8.tile([128, KT8, NB], fp8, tag="x8", name="x8")
                    nc.gpsimd.dma_start(nat8, x8_d[sname][b])

                ph = [ps_h.tile([128, NB], f32, tag="ph", name="ph") for _ in range(HH_T)]
                hsb = [hp.tile([128, NB], bf16, tag="h", name="h") for _ in range(HH_T)]
                for hh in range(HH_T):
                    for k in range(KTB):
                        nc.tensor.matmul(
                            ph[hh],
                            w1b_sb[sname][:, hh, k, :],
                            nat[:, k, :],
                            start=(k == 0),
                            stop=False,
                        )
                        tick()
                    for j in range(KT8 // 2):
                        nc.tensor.matmul(
                            ph[hh],
                            w18_sb[sname][:, hh, 2 * j : 2 * j + 2, :],
                            nat8[:, 2 * j : 2 * j + 2, :],
                            start=False,
                            stop=(j == KT8 // 2 - 1),
                            perf_mode=DR,
                        )
                        tick()
                    nc.scalar.activation(
                        hsb[hh], ph[hh], AF.Relu, bias=b1_sb[sname][:, hh : hh + 1]
                    )

                cell = {}

                def l2a(sname=sname, hsb=hsb, cell=cell):
                    puv = ps_uv.tile([O, NB], f32, tag="puv", name="puv")
                    cell["puv"] = puv
                    nc.tensor.matmul(
                        puv, w2_sb[sname][:, 0, :], hsb[0], start=True, stop=False
                    )

                def l2b(sname=sname, hsb=hsb, cell=cell, stash=stash):
                    puv = cell["puv"]
                    nc.tensor.matmul(
                        puv, w2_sb[sname][:, 1, :], hsb[1], start=False, stop=True
                    )
                    usb = uvp.tile([O, NB], bf16, tag="uv", name="uv")
                    nc.vector.tensor_scalar_add(usb, puv, b2_sb[sname])
                    stash[sname] = usb

                after_mms(3, l2a)
                after_mms(6, l2b)

            def dot(b=b, stash=stash):
                prod = uvp.tile([O, NB], bf16, tag="prod", name="prod")
                nc.vector.tensor_mul(prod, stash["u"], stash["i"])
                pd = ps_d.tile([1, NB], f32, tag="pd", name="pd")
                nc.tensor.matmul(pd, ones_sb, prod, start=True, stop=True)
                s_blk = sp.tile([1, NB], f32, tag="sblk", name="s_blk")
                nc.scalar.activation(s_blk, pd, AF.Sigmoid)
                # Final block's output goes out on the (long-idle) sync
                # ring for the lightest completion path at kernel end.
                eng = nc.sync if b == NBLK - 1 else nc.gpsimd
                eng.dma_start(out[b * NB : (b + 1) * NB], s_blk)

            after_mms(9, dot)

        flush()

    nc.compile()
    return nc


def _pack_x(x):
    """[N_FULL, D] f32 -> per-core ([NBLK,128,KTB,NB] bf16, [NBLK,128,KT8,NB] fp8).

    packed_b[c][b, p, k, n] = x[c*NC_ROWS + b*NB + n, k*128 + p]        (k < KTB)
    packed_8[c][b, p, j, n] = x[c*NC_ROWS + b*NB + n, (KTB+j)*128 + p]  (fp8)
    """
    xf = np.asarray(x, dtype=np.float32)
    xb = xf[:, : KTB * 128].astype(ml_dtypes.bfloat16)
    x8 = xf[:, KTB * 128 :].astype(ml_dtypes.float8_e4m3fn)
    xb = xb.reshape(N_CORES, NBLK, NB, KTB, 128)
    x8 = x8.reshape(N_CORES, NBLK, NB, KT8, 128)
    return (
        [np.ascontiguousarray(xb[c].transpose(0, 3, 2, 1)) for c in range(N_CORES)],
        [np.ascontiguousarray(x8[c].transpose(0, 3, 2, 1)) for c in range(N_CORES)],
    )


def _pack_w1(w1):
    """[D, H] -> hh-major ([128, HH_T, KTB, 128] bf16, [128, HH_T, KT8, 128] fp8).

    element (p, a, k, m) = W1SCALE * w1[k*128 + p, a*128 + m]
    """
    wf = np.asarray(w1, dtype=np.float32) * np.float32(W1SCALE)
    wb = wf[: KTB * 128].astype(ml_dtypes.bfloat16)
    w8 = wf[KTB * 128 :].astype(ml_dtypes.float8_e4m3fn)
    wb = wb.reshape(KTB, 128, HH_T, 128).transpose(1, 2, 0, 3)
    w8 = w8.reshape(KT8, 128, HH_T, 128).transpose(1, 2, 0, 3)
    return np.ascontiguousarray(wb), np.ascontiguousarray(w8)


def _pack_w2(w2):
    """[H, O] -> [128, HH_T, O] bf16, scaled by 1/W1SCALE."""
    wb = (np.asarray(w2, dtype=np.float32) / np.float32(W1SCALE)).astype(
        ml_dtypes.bfloat16
    )
    return np.ascontiguousarray(wb.reshape(HH_T, 128, O).transpose(1, 0, 2))


def _pack_cst(b1u, b1i, b2u, b2i):
    """[128, 6] f32: W1SCALE*b1u (2 cols), W1SCALE*b1i (2 cols), b2u, b2i."""
    cst = np.zeros((128, 6), dtype=np.float32)
    cst[:, 0:2] = np.float32(W1SCALE) * b1u.reshape(2, 128).T
    cst[:, 2:4] = np.float32(W1SCALE) * b1i.reshape(2, 128).T
    cst[: b2u.shape[0], 4] = b2u
    cst[: b2i.shape[0], 5] = b2i
    return cst


def _get_program():
    global _PROGRAM
    if _PROGRAM is None:
        _PROGRAM = _build_program()
    return _PROGRAM


def kernel(
    user_origin_emb,
    item_origin_emb,
    u_w1,
    u_b1,
    u_w2,
    u_b2,
    i_w1,
    i_b1,
    i_w2,
    i_b2,
):
    global LAST_RESULTS
    from concourse.bass_utils import run_bass_kernel_spmd

    xub, xu8 = _pack_x(user_origin_emb)
    xvb, xv8 = _pack_x(item_origin_emb)
    w1ub, w1u8 = _pack_w1(u_w1)
    w1ib, w1i8 = _pack_w1(i_w1)
    shared = {
        "w1ub": w1ub,
        "w1u8": w1u8,
        "w1ib": w1ib,
        "w1i8": w1i8,
        "w2u": _pack_w2(u_w2),
        "w2i": _pack_w2(i_w2),
        "cst": _pack_cst(
            np.asarray(u_b1, dtype=np.float32),
            np.asarray(i_b1, dtype=np.float32),
            np.asarray(u_b2, dtype=np.float32),
            np.asarray(i_b2, dtype=np.float32),
        ),
        "ones": np.ones((O, 1), dtype=ml_dtypes.bfloat16),
    }

    nc = _get_program()
    in_maps = [
        {"xub": xub[c], "xu8": xu8[c], "xvb": xvb[c], "xv8": xv8[c], **shared}
        for c in range(N_CORES)
    ]
    res = run_bass_kernel_spmd(nc, in_maps, core_ids=list(range(N_CORES)), trace=TRACE)
    LAST_RESULTS = res
    return np.concatenate([r["out"] for r in res.results], axis=0)
